# revision 16
# baseline (speedup 1.0000x reference)
"""Trainium2 Bass kernel for BasicEuclideanDistModel log-likelihood.

result = beta*E - sum_e ||z0[u]-z0[v] + (v0[u]-v0[v])*t_e + eps||
         - dt * sum_{p,j} exp(beta - ||dz_p + dv_p*t_j + eps||)

Strategy (8 NeuronCores, data-parallel over events and sampled pairs):
- Events sharded 1/8 per core, sorted by u, cut into 128-event blocks whose
  u-span fits a 128-node window [w_b, w_b+128).
- u-side table rows come from a PE one-hot expand: host provides a fp16
  one-hot [128, 128] per block (row = u - w_b) and the per-block table
  window twin[:, 4b:4b+4] = table[w_b + p, f]; one matmul per block
  accumulates z0x/z0y/v0x/v0y for all 128 events into PSUM (no gather).
- v-side rows fetched with gpsimd.dma_gather from the packed [25001, 64]
  f32 table (4 nodes per 256B block, idx = v//4); the 4-way slot select is
  done with host-provided {0,1} masks (v%4 one-hot), fused into the DVE
  pipeline. This halves the SWDGE descriptor count, which is the kernel's
  roofline (~3ns per gathered index).
- Pairs keep the (u%4, v%4) bucket gather scheme; the Riemann sum uses
  exp(beta - sqrt((dzx+dvx t)^2 + (dzy+dvy t)^2)) evaluated per column.
- Dummy padding entries (u=v=w_b, t=0) contribute exactly eps*sqrt(2)
  (events) / exp(beta - eps*sqrt(2)) per time step (pairs); corrected on
  host. Per-core partial sums [128, 2] are combined on host.
"""
import os as _os
import numpy as np

N_POINTS = 100000
N_RIEMANN = 128
EPS = 1e-6
NON_EVENT_W = 1.0
N_CORES = 8
NBLK = N_POINTS // 4 + 1      # 25000 packed v-table blocks + 1 zero pad
SCRATCH = int(_os.environ.get("KERNEL_SCRATCH", "16384"))
P = 128
GB = int(_os.environ.get("KERNEL_GB", "64"))
VCH = int(_os.environ.get("KERNEL_VCH", "1024"))

SP = bool(int(_os.environ.get("KERNEL_SP", "1")))

_cache = {}


def _build(n_blocks, pair_cols_per_bucket, queues=(0, 1, 2, 3)):
    """Build + compile the SPMD graph. Shapes identical across cores.

    n_blocks: number of 128-event blocks (multiple of 16).
    pair_cols_per_bucket: tuple of 16 ints, 128-pair columns per bucket.
    """
    import concourse.bacc as bacc
    import concourse.mybir as mybir
    import concourse.tile as tile

    f32 = mybir.dt.float32
    f16 = mybir.dt.float16
    i16 = mybir.dt.int16
    AX = mybir.AxisListType
    OP = mybir.AluOpType
    ACT = mybir.ActivationFunctionType

    NQS = len(queues)
    NEV = n_blocks * P
    n_pair_cols = sum(pair_cols_per_bucket)
    NPR = n_pair_cols * P

    nc = bacc.Bacc(num_swdge_queues=1 + max(queues),
                   dynamic_dma_scratch_size=SCRATCH)
    table_e = nc.declare_dram_parameter("table", [NBLK, 64], f32, isOutput=False)
    oneh_e = nc.declare_dram_parameter("oneh", [P, NEV], f16, isOutput=False)
    twin_e = nc.declare_dram_parameter("twin", [P, 4 * n_blocks], f16,
                                       isOutput=False)
    te_e = nc.declare_dram_parameter("te", [P, n_blocks], f32, isOutput=False)
    vmask_e = nc.declare_dram_parameter("vmask", [P, 4 * n_blocks], f32,
                                        isOutput=False)
    vbl_e = nc.declare_dram_parameter("vblk", [P, NEV // 16], i16, isOutput=False)
    pu_e = nc.declare_dram_parameter("publk", [P, NPR // 16], i16, isOutput=False)
    pv_e = nc.declare_dram_parameter("pvblk", [P, NPR // 16], i16, isOutput=False)
    t2_e = nc.declare_dram_parameter("t2d", [P, N_RIEMANN], f32, isOutput=False)
    bt_e = nc.declare_dram_parameter("betac", [P, 1], f32, isOutput=False)
    tn_e = nc.declare_dram_parameter("tinyc", [P, 1], f32, isOutput=False)
    out_e = nc.declare_dram_parameter("out", [P, 2], f32, isOutput=True)

    n_groups = (n_blocks + GB - 1) // GB

    from contextlib import ExitStack
    with tile.TileContext(nc) as tc:
        with tc.tile_pool(name="persist", bufs=1) as pp, \
             tc.tile_pool(name="ps", bufs=1, space="PSUM") as ps, \
             ExitStack() as late:
            pub = pp.tile([P, NPR // 16], i16)
            pvb = pp.tile([P, NPR // 16], i16)
            c1st = min(64, NPR // 16)
            nc.sync.dma_start(out=pub[:, :c1st], in_=pu_e[:, :c1st])
            nc.sync.dma_start(out=pvb[:, :c1st], in_=pv_e[:, :c1st])
            if NPR // 16 > c1st:
                nc.sync.dma_start(out=pub[:, c1st:], in_=pu_e[:, c1st:])
                nc.sync.dma_start(out=pvb[:, c1st:], in_=pv_e[:, c1st:])
            t2d = pp.tile([P, N_RIEMANN], f32)
            nc.sync.dma_start(out=t2d[:], in_=t2_e[:])
            bcol = pp.tile([P, 1], f32)
            nc.sync.dma_start(out=bcol[:], in_=bt_e[:])
            tinyc = pp.tile([P, 1], f32)
            nc.sync.dma_start(out=tinyc[:], in_=tn_e[:])
            vbl = pp.tile([P, NEV // 16], i16)
            v1st = min(GB * P // 16, NEV // 16)
            nc.sync.dma_start(out=vbl[:, :v1st], in_=vbl_e[:, :v1st])
            if NEV // 16 > v1st:
                nc.sync.dma_start(out=vbl[:, v1st:], in_=vbl_e[:, v1st:])
            te = pp.tile([P, n_blocks], f32)
            nc.sync.dma_start(out=te[:], in_=te_e[:])
            vmask = pp.tile([P, 4 * n_blocks], f32)
            nc.sync.dma_start(out=vmask[:], in_=vmask_e[:])
            twin = pp.tile([P, 4 * n_blocks], f16)
            nc.sync.dma_start(out=twin[:], in_=twin_e[:])

            KBC = 8
            n_pgroups = (n_pair_cols + KBC - 1) // KBC
            acc_ne = pp.tile([P, n_pgroups], f32)
            acc_ev = pp.tile([P, 1], f32)
            qtile = pp.tile([P, n_blocks], f32)
            udata = pp.tile([P, 4 * n_blocks], f32)

            goh = late.enter_context(tc.tile_pool(name="goh", bufs=2))
            gev = late.enter_context(tc.tile_pool(name="gev", bufs=4))
            # prefetch: first HG event groups' v-gathers issued before pairs
            HG = 2
            pre_gve = {}
            qi = 0
            for g in range(min(HG, n_groups)):
                b0 = g * GB
                b1 = min(b0 + GB, n_blocks)
                gve = gev.tile([P, GB, 64], f32, tag="gve", name="gve")
                pre_gve[g] = gve
                e0, e1 = b0 * P, b1 * P
                j = 0
                while e0 + j < e1:
                    w = min(VCH, e1 - (e0 + j))
                    s0 = e0 + j
                    nc.gpsimd.dma_gather(
                        out_ap=gve[:, j // P:(j + w) // P, :], in_ap=table_e[:],
                        idxs_ap=vbl[:, s0 // 16:(s0 + w) // 16],
                        num_idxs=w, num_idxs_reg=w, elem_size=64,
                        single_packet=SP, queue_num=queues[qi % NQS])
                    j += w
                    qi += 1

            # ---- pairs: chunked gathers + per-chunk bucket extraction ----
            gpr = late.enter_context(tc.tile_pool(name="gpr", bufs=5))
            dzx = pp.tile([P, n_pair_cols], f32)
            dzy = pp.tile([P, n_pair_cols], f32)
            dvx = pp.tile([P, n_pair_cols], f32)
            dvy = pp.tile([P, n_pair_cols], f32)
            bnds = []
            pc0 = 0
            for b in range(16):
                ncols = pair_cols_per_bucket[b]
                if ncols:
                    bnds.append((pc0, pc0 + ncols, b))
                pc0 += ncols
            PCH = 1024
            CCH = PCH // P
            for ci, q0 in enumerate(range(0, NPR, PCH)):
                q1 = min(q0 + PCH, NPR)
                nq = q1 - q0
                c0, c1 = q0 // P, q1 // P
                gut = gpr.tile([P, CCH, 64], f32, tag="gpu", name="gut")
                gvt = gpr.tile([P, CCH, 64], f32, tag="gpv", name="gvt")
                nc.gpsimd.dma_gather(
                    out_ap=gut[:, :c1 - c0, :], in_ap=table_e[:],
                    idxs_ap=pub[:, q0 // 16:q1 // 16],
                    num_idxs=nq, num_idxs_reg=nq, elem_size=64,
                    single_packet=SP,
                    queue_num=queues[(2 * ci) % NQS])
                nc.gpsimd.dma_gather(
                    out_ap=gvt[:, :c1 - c0, :], in_ap=table_e[:],
                    idxs_ap=pvb[:, q0 // 16:q1 // 16],
                    num_idxs=nq, num_idxs_reg=nq, elem_size=64,
                    single_packet=SP,
                    queue_num=queues[(2 * ci + 1) % NQS])
                for (pb0, pb1, b) in bnds:
                    a0 = max(pb0, c0)
                    a1 = min(pb1, c1)
                    if a0 >= a1:
                        continue
                    ou, ov = 16 * (b // 4), 16 * (b % 4)
                    lo = slice(a0 - c0, a1 - c0)
                    gl = slice(a0, a1)
                    nc.vector.scalar_tensor_tensor(
                        out=dzx[:, gl], in0=gut[:, lo, ou], scalar=EPS,
                        in1=gvt[:, lo, ov], op0=OP.add, op1=OP.subtract)
                    nc.vector.scalar_tensor_tensor(
                        out=dzy[:, gl], in0=gut[:, lo, ou + 1], scalar=EPS,
                        in1=gvt[:, lo, ov + 1], op0=OP.add, op1=OP.subtract)
                    nc.vector.tensor_tensor(out=dvx[:, gl],
                                            in0=gut[:, lo, ou + 2],
                                            in1=gvt[:, lo, ov + 2],
                                            op=OP.subtract)
                    nc.vector.tensor_tensor(out=dvy[:, gl],
                                            in0=gut[:, lo, ou + 3],
                                            in1=gvt[:, lo, ov + 3],
                                            op=OP.subtract)
            # completed square: d^2 = cq*(t+s)^2 + r with
            # h = dz.dv, s = h/max(cq,tiny), r = a - h*s, a = |dz|^2
            aq = pp.tile([P, n_pair_cols], f32)
            hq = pp.tile([P, n_pair_cols], f32)
            cq = pp.tile([P, n_pair_cols], f32)
            sq_ = pp.tile([P, n_pair_cols], f32)
            rq = pp.tile([P, n_pair_cols], f32)
            w1 = pp.tile([P, n_pair_cols], f32)
            nc.vector.tensor_tensor(out=aq[:], in0=dzx[:], in1=dzx[:],
                                    op=OP.mult)
            nc.vector.tensor_tensor(out=w1[:], in0=dzy[:], in1=dzy[:],
                                    op=OP.mult)
            nc.vector.tensor_tensor(out=aq[:], in0=aq[:], in1=w1[:],
                                    op=OP.add)
            nc.vector.tensor_tensor(out=hq[:], in0=dzx[:], in1=dvx[:],
                                    op=OP.mult)
            nc.vector.tensor_tensor(out=w1[:], in0=dzy[:], in1=dvy[:],
                                    op=OP.mult)
            nc.vector.tensor_tensor(out=hq[:], in0=hq[:], in1=w1[:],
                                    op=OP.add)
            nc.vector.tensor_tensor(out=cq[:], in0=dvx[:], in1=dvx[:],
                                    op=OP.mult)
            nc.vector.tensor_tensor(out=w1[:], in0=dvy[:], in1=dvy[:],
                                    op=OP.mult)
            nc.vector.tensor_tensor(out=cq[:], in0=cq[:], in1=w1[:],
                                    op=OP.add)
            nc.vector.tensor_tensor(out=w1[:], in0=cq[:],
                                    in1=tinyc[:].to_broadcast(cq.shape),
                                    op=OP.add)
            nc.vector.reciprocal(out=w1[:], in_=w1[:])
            nc.vector.tensor_tensor(out=sq_[:], in0=hq[:], in1=w1[:],
                                    op=OP.mult)
            nc.vector.tensor_tensor(out=rq[:], in0=hq[:], in1=sq_[:],
                                    op=OP.mult)
            nc.vector.tensor_tensor(out=rq[:], in0=aq[:], in1=rq[:],
                                    op=OP.subtract)
            wk = late.enter_context(tc.tile_pool(name="wk", bufs=2))
            wp = late.enter_context(tc.tile_pool(name="wp", bufs=2))
            KB = 8
            for g in range(n_pgroups):
                k0 = g * KB
                k1 = min(k0 + KB, n_pair_cols)
                yb = wp.tile([P, KB, N_RIEMANN], f32, tag="yb")
                for k in range(k0, k1):
                    j = k - k0
                    nc.vector.tensor_tensor(
                        out=yb[:, j, :], in0=t2d[:],
                        in1=sq_[:, k:k + 1].to_broadcast([P, N_RIEMANN]),
                        op=OP.add)
                    nc.vector.tensor_tensor(out=yb[:, j, :], in0=yb[:, j, :],
                                            in1=yb[:, j, :], op=OP.mult)
                    nc.scalar.activation(yb[:, j, :], yb[:, j, :], ACT.Sqrt,
                                         scale=cq[:, k:k + 1],
                                         bias=rq[:, k:k + 1])
                nc.scalar.activation(
                    yb[:, :k1 - k0, :], yb[:, :k1 - k0, :], ACT.Exp,
                    bias=bcol[:], scale=-1.0,
                    accum_out=acc_ne[:, g:g + 1])

            # ---- events: PE one-hot expand (u) + dma_gather (v) ----
            for g in range(n_groups):
                b0 = g * GB
                b1 = min(b0 + GB, n_blocks)
                nb = b1 - b0
                # u-side: one-hot DMA + one matmul per block -> PSUM
                oh = goh.tile([P, GB * P], f16, tag="oh")
                nc.sync.dma_start(out=oh[:, :nb * P],
                                  in_=oneh_e[:, b0 * P:b1 * P])
                pt = ps.tile([P, 4 * GB], f32, tag=f"pb{g % 4}")
                for j in range(nb):
                    b = b0 + j
                    nc.tensor.matmul(
                        pt[:, 4 * j:4 * j + 4],
                        oh[:, j * P:(j + 1) * P],
                        twin[:, 4 * b:4 * b + 4],
                        start=True, stop=True)
                nc.vector.tensor_copy(udata[:, 4 * b0:4 * b1], pt[:, :4 * nb])
                # v-side: gathers into gv (first HG groups prefetched)
                if g in pre_gve:
                    gve = pre_gve[g]
                else:
                    gve = gev.tile([P, GB, 64], f32, tag="gve", name="gve")
                    e0 = b0 * P
                    e1 = b1 * P
                    j = 0
                    while e0 + j < e1:
                        w = min(VCH, e1 - (e0 + j))
                        s0 = e0 + j
                        nc.gpsimd.dma_gather(
                            out_ap=gve[:, j // P:(j + w) // P, :],
                            in_ap=table_e[:],
                            idxs_ap=vbl[:, s0 // 16:(s0 + w) // 16],
                            num_idxs=w, num_idxs_reg=w, elem_size=64,
                            single_packet=SP, queue_num=queues[qi % NQS])
                        j += w
                        qi += 1
                # extract v features via masks and accumulate event distances
                uv = wk.tile([P, 8, GB], f32, tag="uv")
                for f in range(4):
                    vm0 = vmask[:, 4 * b0 + 0:4 * (b1 - 1) + 1:4]
                    nc.vector.tensor_tensor(out=uv[:, f, :nb],
                                            in0=vm0, in1=gve[:, :nb, 0 + f],
                                            op=OP.mult)
                    for s in range(1, 4):
                        vms = vmask[:, 4 * b0 + s:4 * (b1 - 1) + s + 1:4]
                        tmp = wk.tile([P, GB], f32, tag="tmp")
                        nc.vector.tensor_tensor(out=tmp[:, :nb], in0=vms,
                                                in1=gve[:, :nb, 16 * s + f],
                                                op=OP.mult)
                        nc.vector.tensor_tensor(out=uv[:, f, :nb],
                                                in0=uv[:, f, :nb],
                                                in1=tmp[:, :nb], op=OP.add)
                # dz = u - v + eps ; m = dz + dv*t ; q = mx^2 + my^2
                for f in range(4):
                    if f < 2:
                        nc.vector.scalar_tensor_tensor(
                            out=uv[:, 4 + f, :nb],
                            in0=udata[:, 4 * b0 + f:4 * (b1 - 1) + f + 1:4],
                            scalar=EPS, in1=uv[:, f, :nb],
                            op0=OP.add, op1=OP.subtract)
                    else:
                        nc.vector.tensor_tensor(
                            out=uv[:, 4 + f, :nb],
                            in0=udata[:, 4 * b0 + f:4 * (b1 - 1) + f + 1:4],
                            in1=uv[:, f, :nb], op=OP.subtract)
                teg = te[:, b0:b1]
                mx = wk.tile([P, GB], f32, tag="emx")
                nc.vector.tensor_tensor(out=mx[:, :nb], in0=uv[:, 6, :nb],
                                        in1=teg, op=OP.mult)
                nc.vector.tensor_tensor(out=mx[:, :nb], in0=mx[:, :nb],
                                        in1=uv[:, 4, :nb], op=OP.add)
                my = wk.tile([P, GB], f32, tag="emy")
                nc.vector.tensor_tensor(out=my[:, :nb], in0=uv[:, 7, :nb],
                                        in1=teg, op=OP.mult)
                nc.vector.tensor_tensor(out=my[:, :nb], in0=my[:, :nb],
                                        in1=uv[:, 5, :nb], op=OP.add)
                nc.vector.tensor_tensor(out=mx[:, :nb], in0=mx[:, :nb],
                                        in1=mx[:, :nb], op=OP.mult)
                nc.vector.tensor_tensor(out=my[:, :nb], in0=my[:, :nb],
                                        in1=my[:, :nb], op=OP.mult)
                nc.vector.tensor_tensor(out=qtile[:, b0:b1], in0=mx[:, :nb],
                                        in1=my[:, :nb], op=OP.add)

            nc.scalar.activation(qtile[:], qtile[:], ACT.Sqrt,
                                 accum_out=acc_ev[:])
            res = pp.tile([P, 2], f32)
            nc.vector.tensor_copy(res[:, 0:1], acc_ev[:])
            nc.vector.tensor_reduce(res[:, 1:2], acc_ne[:], axis=AX.X,
                                    op=OP.add)
            nc.sync.dma_start(out=out_e[:], in_=res[:])

    nc.compile()
    return nc


def _wrap16(blk):
    """[N] int16 block ids -> [128, N//16] dma_gather index layout."""
    w = blk.reshape(-1, 16).T          # [16, N//16]
    return np.tile(w, (8, 1)).astype(np.int16)


def _plane(arr, dtype=np.float32):
    """[N] -> [128, N//128] with event i=(c*128+p) at [p, c]."""
    return np.ascontiguousarray(arr.reshape(-1, 128).T).astype(dtype)


def _bucketize(u, v):
    key = (u % 4) * 4 + (v % 4)
    order = np.argsort(key, kind="stable")
    counts = np.bincount(key, minlength=16)
    return order, counts


def _cut_blocks(us):
    """Greedy cut of sorted u values into 128-event blocks with span < 128.

    Returns list of (start, end, w) with end-start <= 128 and
    us[start:end] all in [w, w+128).
    """
    blocks = []
    n = len(us)
    i = 0
    while i < n:
        w = int(us[i])
        j = min(i + P, n)
        # find first k in (i, j) with us[k] >= w + 128
        hi = np.searchsorted(us[i:j], w + P, side="left")
        j = i + int(hi)
        blocks.append((i, j, w))
        i = j
    return blocks


def kernel(beta, z0, v0, a0, u, v, event_times, pair_u, pair_v, t0, tn):
    assert not np.any(np.asarray(a0)), "kernel assumes a0 == 0"
    beta = np.asarray(beta, np.float32)
    z0 = np.asarray(z0, np.float32)
    v0 = np.asarray(v0, np.float32)
    u = np.asarray(u).astype(np.int64)
    v = np.asarray(v).astype(np.int64)
    event_times = np.asarray(event_times, np.float32)
    pair_u = np.asarray(pair_u).astype(np.int64)
    pair_v = np.asarray(pair_v).astype(np.int64)
    t0f = float(np.asarray(t0))
    tnf = float(np.asarray(tn))
    b = float(beta.reshape(-1)[0])
    E = u.shape[0]
    NPAIR = pair_u.shape[0]
    ev_sh = E // N_CORES
    pr_sh = NPAIR // N_CORES

    # packed padded v-table: [25000, 64]; node n at block n//4, slot n%4
    tbl = np.zeros((NBLK * 4, 16), np.float32)
    tbl[:N_POINTS, 0:2] = z0
    tbl[:N_POINTS, 2:4] = v0
    tbl = np.ascontiguousarray(tbl.reshape(NBLK, 64))

    # dense f32 feature table for the u-side windows
    tabf = np.zeros((N_POINTS + P, 4), np.float32)
    tabf[:N_POINTS, 0:2] = z0
    tabf[:N_POINTS, 2:4] = v0

    # ---- per-core host prep ----
    cores = []
    pr_orders, pr_counts = [], []
    max_blocks = 0
    for c in range(N_CORES):
        s = slice(c * ev_sh, (c + 1) * ev_sh)
        uu, vv, tt = u[s], v[s], event_times[s]
        order = np.argsort(uu, kind="stable")
        us, vs, ts = uu[order], vv[order], tt[order]
        blocks = _cut_blocks(us)
        cores.append((us, vs, ts, blocks))
        max_blocks = max(max_blocks, len(blocks))
        s = slice(c * pr_sh, (c + 1) * pr_sh)
        o, cnt = _bucketize(pair_u[s], pair_v[s])
        pr_orders.append(o)
        pr_counts.append(cnt)
    n_blocks = max_blocks
    NEV = n_blocks * P

    pr_counts = np.stack(pr_counts)
    pr_cap = (pr_counts.max(axis=0) + P - 1) // P * P
    pr_cols = tuple(int(x) for x in pr_cap // P)
    NPR = int(pr_cap.sum())

    key = (n_blocks, pr_cols)
    if key not in _cache:
        _cache[key] = _build(n_blocks, pr_cols)
    nc = _cache[key]

    dt = (tnf - t0f) / N_RIEMANN
    ts_r = (t0f + (np.arange(N_RIEMANN, dtype=np.float32) / N_RIEMANN)
            * (tnf - t0f)).astype(np.float32)
    t2d = np.tile(ts_r[None, :], (P, 1))

    in_maps = []
    n_ev_dummy = np.zeros(N_CORES, np.int64)
    n_pr_dummy = np.zeros(N_CORES, np.int64)
    for c in range(N_CORES):
        us, vs, ts, blocks = cores[c]
        # padded per-block event arrays
        ub = np.zeros(NEV, np.int64)
        vb = np.zeros(NEV, np.int64)
        tb = np.zeros(NEV, np.float32)
        urel = np.zeros(NEV, np.int16)
        wvec = np.zeros(n_blocks, np.int64)
        for bi, (i0, i1, w) in enumerate(blocks):
            nreal = i1 - i0
            o = bi * P
            ub[o:o + nreal] = us[i0:i1]
            vb[o:o + nreal] = vs[i0:i1]
            tb[o:o + nreal] = ts[i0:i1]
            urel[o:o + nreal] = (us[i0:i1] - w).astype(np.int16)
            # pad: u=v=w, t=0 -> d = eps*sqrt(2)
            ub[o + nreal:o + P] = w
            vb[o + nreal:o + P] = w
            wvec[bi] = w
        # pad blocks beyond len(blocks): u=v=0 -> d = eps*sqrt(2)
        n_ev_dummy[c] = NEV - ev_sh

        # one-hot [128, NEV] fp16: row urel, col event
        oneh = np.zeros((P, NEV), np.float16)
        oneh[urel, np.arange(NEV)] = np.float16(1.0)
        # table windows [128, 4*n_blocks] fp16
        twin = tabf[wvec[:, None] + np.arange(P)[None, :], :]  # [nb,128,4]
        twin = np.ascontiguousarray(
            twin.transpose(1, 0, 2).reshape(P, 4 * n_blocks)).astype(np.float16)
        # v masks [128, 4*n_blocks] f32: vmask[p, 4b+s] = (v % 4 == s)
        vsl = (vb % 4).reshape(n_blocks, P)                   # [nb, 128]
        vmask = np.zeros((P, n_blocks, 4), np.float32)
        bi_ix = np.repeat(np.arange(n_blocks), P)
        p_ix = np.tile(np.arange(P), n_blocks)
        vmask[p_ix, bi_ix, vsl.reshape(-1)] = 1.0
        vmask = np.ascontiguousarray(vmask.reshape(P, 4 * n_blocks))

        sp = slice(c * pr_sh, (c + 1) * pr_sh)
        pu_, pv_ = pair_u[sp], pair_v[sp]
        o, cnt = pr_orders[c], pr_counts[c]
        pub = np.full(NPR, N_POINTS, np.int64)
        pvb = np.full(NPR, N_POINTS, np.int64)
        off = 0
        pos = 0
        for bk in range(16):
            n = int(cnt[bk])
            idxs = o[pos:pos + n]
            pub[off:off + n] = pu_[idxs]
            pvb[off:off + n] = pv_[idxs]
            pos += n
            off += int(pr_cap[bk])
        n_pr_dummy[c] = NPR - pr_sh

        in_maps.append({
            "table": tbl,
            "oneh": oneh,
            "twin": twin,
            "te": _plane(tb),
            "vmask": vmask,
            "vblk": _wrap16(vb // 4),
            "publk": _wrap16(pub // 4),
            "pvblk": _wrap16(pvb // 4),
            "t2d": t2d,
            "betac": np.full((P, 1), b, np.float32),
            "tinyc": np.full((P, 1), 1e-16, np.float32),
        })

    import os
    trace = bool(os.environ.get("KERNEL_TRACE"))
    if trace:
        try:
            import sys, types
            if "antenv.axon_hooks" not in sys.modules:
                mod = types.ModuleType("antenv.axon_hooks")
                mod._hook = None
                mod.set_axon_ntff_profile_hook = lambda h: setattr(mod, "_hook", h)
                mod.get_axon_ntff_profile_hook = lambda: mod._hook
                import antenv
                antenv.axon_hooks = mod
                sys.modules["antenv.axon_hooks"] = mod
                from trn_agent_boot.trn_boot import _ntff_profile_via_ctypes
                hk = _ntff_profile_via_ctypes("/opt/axon/libaxon_pjrt.so")
                if hk is not None:
                    mod.set_axon_ntff_profile_hook(hk)
        except Exception:
            trace = False
    from concourse.bass_utils import run_bass_kernel_spmd
    r = run_bass_kernel_spmd(nc, in_maps, core_ids=list(range(N_CORES)),
                             trace=trace)
    globals()["LAST_EXEC_NS"] = r.exec_time_ns

    ev_sum = 0.0
    ne_sum = 0.0
    for c in range(N_CORES):
        out = r.results[c]["out"].astype(np.float64)
        ev_sum += out[:, 0].sum()
        ne_sum += out[:, 1].sum()

    # dummy corrections (u=v => diff = (eps, eps))
    d_dummy = np.sqrt(2.0) * EPS
    ev_sum -= float(n_ev_dummy.sum()) * d_dummy
    ne_sum -= float(n_pr_dummy.sum()) * N_RIEMANN * np.exp(b - d_dummy)

    global DEBUG_PARTS
    DEBUG_PARTS = (ev_sum, ne_sum)
    result = b * E - ev_sum - NON_EVENT_W * ne_sum * dt
    return np.float32(result)
